# revision 1
# baseline (speedup 1.0000x reference)
"""Bass/Trainium2 kernel for DegreeOnlyFiltration (segment max + gather-divide).

Contract: kernel(**inputs) takes FULL inputs (node_deg [N] f32, sample_pos
[G+1] i32 CSR boundaries) and returns the FULL output node_deg / seg_max.

Strategy (per the sharding hint): segments are contiguous; the expected input
has uniform boundaries (sample_pos = arange(G+1) * W).  We shard node_deg by
whole segments across the 8 NeuronCores (pure data parallel, no cross-core
traffic).  On each core: view the shard as [segs_per_core, W], tile into
[128, W-chunk] SBUF tiles (one segment per partition row), reduce_max along
the free axis, reciprocal, then a per-partition-scalar multiply, and DMA the
result back out.  Measured ~52.5 us on HW (pure DMA roofline ~39 us + ~11 us
fixed NEFF preamble/completion overhead; all 16 SDMA engines >97% busy).
"""

import os

import numpy as np

import concourse.bacc as bacc
import concourse.mybir as mybir
import concourse.tile as tile
from concourse.bass_utils import run_bass_kernel_spmd

N_CORES = 8
P = 128  # SBUF partitions

# Populated after each traced run (test harness reads these).
LAST_EXEC_TIME_NS = None
LAST_RESULTS = None

_NC_CACHE = {}


def _build_uniform_nc(segs_per_core: int, width: int, segs_per_tile: int):
    """SPMD program: x [segs_per_core, width] f32 -> y = x / rowmax(x).

    Each SBUF tile covers P whole segments (one per partition row), split
    column-wise into chunks for fine-grained DMA/compute overlap: partial
    reduce_max per chunk, tensor_max combine, reciprocal, then a
    per-partition-scalar multiply per chunk (alternating DVE/ACT).  Input
    DMAs all issue up front on the SP HWDGE ring; output DMAs issue from the
    scalar engine (the separate ACT HWDGE ring) so the two streams don't
    head-of-line block each other and the SDMA engines round-robin 50/50.
    """
    assert segs_per_core % segs_per_tile == 0
    assert segs_per_tile % P == 0
    rows = segs_per_tile // P  # segments per partition row
    n_tiles = segs_per_core // segs_per_tile
    f32 = mybir.dt.float32

    # Column-chunk plan per tile: big chunks for the bulk (DMA efficiency),
    # tapered chunks for the last tiles (short pipeline tail).
    def chunk_plan(t):
        # 8KB/partition descriptors; the last tile tapers so the final
        # input chunk needs only a short reduce before its output ships.
        # (HBM reads cap at ~388 GB/s per core regardless of descriptor
        # size; writes sustain ~420 — the DMA phase is hardware-pinned.)
        if rows != 1 or width % 2 != 0 or width // 2 < 512:
            return [width]
        if t == n_tiles - 1 and width % 4 == 0 and width // 4 >= 512:
            return [width // 2, width // 4, width // 4]
        return [width // 2] * 2

    def out_plan(cw):
        return [cw]

    nc = bacc.Bacc("TRN2", target_bir_lowering=False, debug=False,
                   num_devices=N_CORES, enable_partition_id=False,
                   enable_asserts=False)
    x = nc.dram_tensor("x", [segs_per_core, width], f32, kind="ExternalInput")
    y = nc.dram_tensor("y", [segs_per_core, width], f32, kind="ExternalOutput")

    with tile.TileContext(nc) as tc:
        with (
            tc.tile_pool(name="pin", bufs=1) as pin,
            tc.tile_pool(name="pout", bufs=1) as pout,
            tc.tile_pool(name="stats", bufs=8 * n_tiles) as pstats,
        ):
            # All input DMAs up front on the SP HWDGE ring: no buffer
            # recycling, no head-of-line blocking behind output DMAs.
            # Distinct tags per chunk -> every chunk gets its own slot.
            tins = []
            for t in range(n_tiles):
                s0 = t * segs_per_tile
                if rows != 1:
                    tin = pin.tile([P, rows * width], f32, tag=f"tin{t}")
                    nc.sync.dma_start(
                        tin[:], x[s0:s0 + segs_per_tile, :].rearrange(
                            "(p r) w -> p (r w)", p=P))
                    tins.append([tin])
                    continue
                chunk = []
                c0 = 0
                for k, cw in enumerate(chunk_plan(t)):
                    tin = pin.tile([P, cw], f32, tag=f"tin{t}.{k}")
                    nc.sync.dma_start(tin[:], x[s0:s0 + P, c0:c0 + cw])
                    chunk.append((c0, cw, tin))
                    c0 += cw
                tins.append(chunk)

            mul_idx = 0
            for t in range(n_tiles):
                s0 = t * segs_per_tile
                if rows != 1:
                    tin = tins[t][0]
                    m = pstats.tile([P, rows], f32, tag="m")
                    nc.vector.reduce_max(
                        m[:], tin[:].rearrange("p (r w) -> p r w", r=rows),
                        axis=mybir.AxisListType.X)
                    r = pstats.tile([P, rows], f32, tag="r")
                    nc.vector.reciprocal(r[:], m[:])
                    tout = pout.tile([P, rows * width], f32, tag=f"tout{t}")
                    for j in range(rows):
                        nc.scalar.mul(tout[:, j * width:(j + 1) * width],
                                      tin[:, j * width:(j + 1) * width],
                                      r[:, j:j + 1])
                    nc.scalar.dma_start(
                        y[s0:s0 + segs_per_tile, :].rearrange(
                            "(p r) w -> p (r w)", p=P), tout[:])
                    continue

                # Partial maxes per chunk, then a combine tree.
                pms = []
                for (c0, cw, tin) in tins[t]:
                    pm = pstats.tile([P, 1], f32, tag="pm")
                    nc.vector.reduce_max(pm[:], tin[:],
                                         axis=mybir.AxisListType.X)
                    pms.append(pm)
                while len(pms) > 1:
                    nxt = []
                    for a, b in zip(pms[::2], pms[1::2]):
                        c = pstats.tile([P, 1], f32, tag="pm")
                        nc.vector.tensor_max(c[:], a[:], b[:])
                        nxt.append(c)
                    if len(pms) % 2:
                        nxt.append(pms[-1])
                    pms = nxt
                r = pstats.tile([P, 1], f32, tag="r")
                nc.vector.reciprocal(r[:], pms[0][:])

                # Emit all muls before any output-DMA issue: the scalar
                # engine is in-order, so a dma_start waiting on the DVE
                # mul's semaphore must not sit ahead of the ACT mul.
                touts = []
                for (c0, cw, tin) in tins[t]:
                    o0 = 0
                    for ow in out_plan(cw):
                        tout = pout.tile([P, ow], f32,
                                         tag=f"tout{t}.{len(touts)}")
                        # Alternate DVE/ACT to balance engine load.
                        if mul_idx % 2 == 0:
                            nc.vector.tensor_scalar_mul(
                                tout[:], tin[:, o0:o0 + ow], r[:])
                        else:
                            nc.scalar.mul(tout[:], tin[:, o0:o0 + ow], r[:])
                        touts.append((c0 + o0, ow, tout))
                        mul_idx += 1
                        o0 += ow
                for (c0, cw, tout) in touts:
                    # Outputs issue from the scalar engine -> the separate
                    # ACT HWDGE ring; the two streams round-robin at the
                    # SDMA engines without head-of-line blocking.
                    nc.scalar.dma_start(y[s0:s0 + P, c0:c0 + cw], tout[:])
    nc.compile()
    return nc


def _uniform_width(sample_pos: np.ndarray, n: int):
    """Return segment width W if boundaries are uniform (pos = arange*W)."""
    if sample_pos[0] != 0 or sample_pos[-1] != n:
        return None
    diffs = np.diff(sample_pos)
    if diffs.size == 0 or np.any(diffs != diffs[0]):
        return None
    return int(diffs[0])


def _host_fallback(node_deg: np.ndarray, sample_pos: np.ndarray) -> np.ndarray:
    """Exact mirror of the reference semantics for non-uniform boundaries."""
    import jax

    with jax.default_device(jax.devices("cpu")[0]):
        import jax.numpy as jnp

        deg = jnp.asarray(node_deg)
        pos = jnp.asarray(sample_pos)
        n = deg.shape[0]
        g = pos.shape[0] - 1
        seg_ids = jnp.searchsorted(pos[1:], jnp.arange(n, dtype=pos.dtype),
                                   side="right")
        seg_max = jax.ops.segment_max(deg, seg_ids, num_segments=g)
        return np.asarray(deg / seg_max[seg_ids])


def kernel(node_deg: np.ndarray, sample_pos: np.ndarray) -> np.ndarray:
    global LAST_EXEC_TIME_NS, LAST_RESULTS

    node_deg = np.asarray(node_deg, dtype=np.float32)
    sample_pos = np.asarray(sample_pos, dtype=np.int32)
    n = node_deg.shape[0]
    g = sample_pos.shape[0] - 1

    width = _uniform_width(sample_pos, n)
    if width is None or g % N_CORES != 0 or (g // N_CORES) % P != 0:
        return _host_fallback(node_deg, sample_pos)

    segs_per_core = g // N_CORES
    # Pick segments per tile so one SBUF tile is ~2 MiB (>=1 MiB DMAs) while
    # keeping whole segments per partition row.
    rows = max(1, min(segs_per_core // P, 4096 // max(1, width)))
    segs_per_tile = P * rows
    while segs_per_core % segs_per_tile != 0:
        rows -= 1
        segs_per_tile = P * rows

    key = (segs_per_core, width, segs_per_tile)
    if key not in _NC_CACHE:
        _NC_CACHE[key] = _build_uniform_nc(*key)
    nc = _NC_CACHE[key]

    shards = node_deg.reshape(N_CORES, segs_per_core, width)
    in_maps = [{"x": shards[c]} for c in range(N_CORES)]

    trace = bool(int(os.environ.get("KERNEL_TRACE", "0")))
    try:
        res = run_bass_kernel_spmd(nc, in_maps, core_ids=list(range(N_CORES)),
                                   trace=trace)
    except Exception:
        if not trace:
            raise
        # Trace post-processing can fail in sandboxes; results still matter.
        res = run_bass_kernel_spmd(nc, in_maps, core_ids=list(range(N_CORES)),
                                   trace=False)
    LAST_EXEC_TIME_NS = res.exec_time_ns
    LAST_RESULTS = res
    out = np.concatenate([res.results[c]["y"].reshape(-1)
                          for c in range(N_CORES)])
    return out.astype(np.float32, copy=False)

